# revision 1
# baseline (speedup 1.0000x reference)
"""Trainium2 Bass kernel for a single dense-transformer attention layer.

Problem (hardcoded): B=1, S=4096, D=2048, H=16 heads, head_dim=128, RoPE,
softmax attention, output projection.  torch-Linear convention: y = x @ W.T.

Sharding: tensor-parallel over heads across 8 NeuronCores.  Each core handles
2 heads: it computes q/k/v projections for its head slice, RoPE, attention,
and a partial output projection (contraction over its 256 head-dims of Wo).
The host sums the 8 partial [S, D] outputs.

Device-side layout choices (everything pre-arranged on host):
  - xT      [128, 16, S]  bf16 : xT[p, o, s] = x[s, o*128+p]      (x transposed)
  - wq/wk/wv[128, 16, 256] bf16: w[p, o, e]  = W[c*256+e, o*128+p] (per core c)
  - wo      [128, 2, D]   bf16 : wo[p, h, n] = Wo[n, (2c+h)*128+p]
  - cosT    [128, S] bf16 : cosT[d, s]  = cos(pos_s * invfreq(d % 64))
  - sinTs   [128, S] bf16 : +sin for d<64, -sin for d>=64 (pre-shifted table
                            so RoPE needs only aligned multiplies + a
                            64-partition-shift copy, done by DMA)

The q/k projections are emitted transposed ([head_dim, s]) so the scores
matmul can contract over head_dim directly; scores are computed transposed
([s_k, s_q]) so exp(scores) feeds the P@V matmul with V in natural layout and
no on-chip transposes anywhere.  The softmax denominator comes from a ones
column vector matmul riding the same exp tiles; normalization is folded into
the PSUM->SBUF copy of the attention output via a broadcast reciprocal.
"""

import os
import numpy as np
import ml_dtypes

import concourse.bacc as bacc
import concourse.bass as bass
import concourse.mybir as mybir
import concourse.tile as tile
from concourse.bass import ds, ts
from concourse.bass_utils import run_bass_kernel_spmd

BF16 = mybir.dt.bfloat16
F32 = mybir.dt.float32
F32R = mybir.dt.float32r
AF = mybir.ActivationFunctionType

S, D, H, HD = 4096, 2048, 16, 128
N_CORES = 8
HPC = H // N_CORES  # heads per core = 2
BASE = 10000.0


def build_nc(seq=S, repeat=1):
    """Build the per-core Bass module (identical program on all 8 cores)."""
    n_strips = seq // 512  # 512-wide s strips
    n_ktiles = seq // 128  # 128-wide k chunks
    DK = D // 128  # 16 contraction chunks for projections

    nc = bacc.Bacc("TRN2", target_bir_lowering=False)

    xT = nc.dram_tensor("xT", [128, seq // 512, DK, 512], BF16, kind="ExternalInput")
    wq = nc.dram_tensor("wq", [128, DK, HPC * HD], BF16, kind="ExternalInput")
    wk = nc.dram_tensor("wk", [128, DK, HPC * HD], BF16, kind="ExternalInput")
    wv = nc.dram_tensor("wv", [128, DK, HPC * HD], BF16, kind="ExternalInput")
    wo = nc.dram_tensor("wo", [128, HPC, D], F32, kind="ExternalInput")
    cosT = nc.dram_tensor("cosT", [128, seq], BF16, kind="ExternalInput")
    sinTs = nc.dram_tensor("sinTs", [128, seq], BF16, kind="ExternalInput")
    outp = nc.dram_tensor("outp", [seq, D], F32, kind="ExternalOutput")

    inv_sqrt_hd = 1.0 / float(np.sqrt(HD))

    with tile.TileContext(nc) as tc:
        from contextlib import ExitStack

        with ExitStack() as ctx:
            cpool = ctx.enter_context(tc.tile_pool(name="const", bufs=1))
            qkpool = ctx.enter_context(tc.tile_pool(name="qk", bufs=1))
            vpool = ctx.enter_context(tc.tile_pool(name="v", bufs=1))

            cos_sb = cpool.tile([128, seq], BF16, tag="cos")
            nc.sync.dma_start(cos_sb[:], cosT[:])
            sin_sb = cpool.tile([128, seq], BF16, tag="sin")
            nc.sync.dma_start(sin_sb[:], sinTs[:])
            wq_sb = cpool.tile([128, DK, HPC * HD], BF16, tag="wq")
            nc.sync.dma_start(wq_sb[:], wq[:])
            wk_sb = cpool.tile([128, DK, HPC * HD], BF16, tag="wk")
            nc.sync.dma_start(wk_sb[:], wk[:])
            wv_sb = cpool.tile([128, DK, HPC * HD], BF16, tag="wv")
            nc.sync.dma_start(wv_sb[:], wv[:])
            wo_sb = cpool.tile([128, HPC, D], F32, tag="wo")
            nc.sync.dma_start(wo_sb[:], wo[:])
            ones_col = cpool.tile([128, 1], F32, tag="ones_col")
            nc.vector.memset(ones_col[:], 1.0)
            ones_row = cpool.tile([1, 128], F32, tag="ones_row")
            nc.vector.memset(ones_row[:], 1.0)

            # persistent per-head activations (bf16)
            q_sb = [qkpool.tile([128, seq], F32, tag=f"q{h}", name=f"q{h}") for h in range(HPC)]
            k_sb = [qkpool.tile([128, seq], F32, tag=f"k{h}", name=f"k{h}") for h in range(HPC)]
            v_sb = [
                vpool.tile([128, n_ktiles, HD], F32, tag=f"v{h}", name=f"v{h}") for h in range(HPC)
            ]

            for _rep in range(repeat):
                # ---------------- Phase 1: QKV projections + RoPE ----------------
                with ExitStack() as p1:
                    xpool = p1.enter_context(tc.tile_pool(name="xchunk", bufs=2))
                    rpool = p1.enter_context(tc.tile_pool(name="rope", bufs=4))
                    ps_qk = p1.enter_context(
                        tc.tile_pool(name="ps_qk", bufs=6, space="PSUM")
                    )
                    ps_v = p1.enter_context(tc.tile_pool(name="ps_v", bufs=2, space="PSUM"))

                    for j in range(n_strips):
                        xc = xpool.tile([128, DK, 512], BF16, tag="xc")
                        nc.sync.dma_start(xc[:], xT[:, j])
                        for h in range(HPC):
                            for w_sb, dst in ((wq_sb, q_sb[h]), (wk_sb, k_sb[h])):
                                ps = ps_qk.tile([128, 512], F32, tag="ps_qk")
                                for o in range(DK):
                                    nc.tensor.matmul(
                                        ps[:],
                                        lhsT=w_sb[:, o, ts(h, HD)],
                                        rhs=xc[:, o, :],
                                        start=(o == 0),
                                        stop=(o == DK - 1),
                                    )
                                # RoPE: dst = ps*cos + shift64(ps*sinTs)
                                mp = rpool.tile([128, 512], BF16, tag="mp")
                                nc.vector.tensor_mul(mp[:], ps[:], sin_sb[:, ts(j, 512)])
                                m = rpool.tile([128, 512], BF16, tag="m")
                                nc.sync.dma_start(m[0:64, :], mp[64:128, :])
                                nc.sync.dma_start(m[64:128, :], mp[0:64, :])
                                tt = rpool.tile([128, 512], F32, tag="tt")
                                nc.vector.tensor_mul(tt[:], ps[:], cos_sb[:, ts(j, 512)])
                                nc.vector.tensor_add(dst[:, ts(j, 512)], tt[:], m[:])
                        for b in range(4):  # v in natural layout, both heads at once
                            sblk = j * 4 + b
                            psv = ps_v.tile([128, HPC * HD], F32, tag="psv")
                            for o in range(DK):
                                nc.tensor.matmul(
                                    psv[:],
                                    lhsT=xc[:, o, ts(b, 128)],
                                    rhs=wv_sb[:, o, :],
                                    start=(o == 0),
                                    stop=(o == DK - 1),
                                )
                            for h in range(HPC):
                                nc.scalar.copy(v_sb[h][:, sblk, :], psv[:, ts(h, HD)])

                # ---------------- Phase 2: attention + output projection --------
                with ExitStack() as p2:
                    epool = p2.enter_context(tc.tile_pool(name="et", bufs=4))
                    opool = p2.enter_context(tc.tile_pool(name="outsb", bufs=3))
                    apool = p2.enter_context(tc.tile_pool(name="attnT", bufs=2))
                    rcpool = p2.enter_context(tc.tile_pool(name="rc", bufs=2))
                    ps_s = p2.enter_context(tc.tile_pool(name="ps_s", bufs=2, space="PSUM"))
                    ps_pv = p2.enter_context(
                        tc.tile_pool(name="ps_pv", bufs=2, space="PSUM")
                    )
                    ps_rs = p2.enter_context(
                        tc.tile_pool(name="ps_rs", bufs=2, space="PSUM")
                    )
                    ps_bc = p2.enter_context(
                        tc.tile_pool(name="ps_bc", bufs=1, space="PSUM")
                    )
                    ps_o = p2.enter_context(tc.tile_pool(name="ps_o", bufs=1, space="PSUM"))

                    for j in range(n_strips):
                        aT = apool.tile([128, HPC, 512], F32, tag="aT")
                        pvs, rss = [], []
                        for h in range(HPC):
                            pv = ps_pv.tile([128, 512], F32, tag="pv", name=f"pv{h}")
                            rs = ps_rs.tile([1, 512], F32, tag="rs", name=f"rs{h}")
                            pvs.append(pv)
                            rss.append(rs)
                            for c in range(n_ktiles):
                                # transposed scores chunk: [s_k 128, s_q 512]
                                ss = ps_s.tile([128, 512], F32, tag="ss")
                                nc.tensor.matmul(
                                    ss[:],
                                    lhsT=k_sb[h][:, ts(c, 128)],
                                    rhs=q_sb[h][:, ts(j, 512)],
                                    start=True,
                                    stop=True,
                                )
                                et = epool.tile([128, 512], F32, tag="et")
                                nc.scalar.activation(
                                    et[:], ss[:], AF.Exp, bias=0.0, scale=inv_sqrt_hd
                                )
                                nc.tensor.matmul(
                                    pv[:],
                                    lhsT=v_sb[h][:, c, :],
                                    rhs=et[:],
                                    start=(c == 0),
                                    stop=(c == n_ktiles - 1),
                                )
                                nc.tensor.matmul(
                                    rs[:],
                                    lhsT=ones_col[:],
                                    rhs=et[:],
                                    start=(c == 0),
                                    stop=(c == n_ktiles - 1),
                                )
                        # normalization for both heads, after both chunk loops so
                        # the 64 exps above form one uninterrupted ACT run
                        for h in range(HPC):
                            rcp = rcpool.tile([1, 512], F32, tag="rcp")
                            nc.vector.reciprocal(rcp[:], rss[h][:])
                            bc = ps_bc.tile([128, 512], F32, tag="bc")
                            nc.tensor.matmul(
                                bc[:], lhsT=ones_row[:], rhs=rcp[:], start=True, stop=True
                            )
                            bcs = rcpool.tile([128, 512], F32, tag="bcs")
                            nc.vector.tensor_copy(bcs[:], bc[:])
                            nc.vector.tensor_mul(aT[:, h, :], pvs[h][:], bcs[:])
                        # output projection for the strip's 4 q-tiles
                        for b in range(4):
                            ob = opool.tile([128, D], F32, tag="ob")
                            for n in range(D // 512):
                                po = ps_o.tile([128, 512], F32, tag="po")
                                for h in range(HPC):
                                    nc.tensor.matmul(
                                        po[:],
                                        lhsT=aT[:, h, ts(b, 128)],
                                        rhs=wo_sb[:, h, ts(n, 512)],
                                        start=(h == 0),
                                        stop=(h == HPC - 1),
                                    )
                                nc.vector.tensor_copy(ob[:, ts(n, 512)], po[:])
                            nc.sync.dma_start(outp[ds((j * 4 + b) * 128, 128), :], ob[:])

    nc.compile()
    return nc


def make_in_maps(hidden_states, Wq, Wk, Wv, Wo, position_ids, seq=S):
    """Host-side prep: transpose/shard/cast inputs into per-core in_maps."""
    bf16 = ml_dtypes.bfloat16
    x = np.asarray(hidden_states, dtype=np.float32).reshape(seq, D)
    # [p, strip, o, 512] so each strip chunk is contiguous per partition
    xT = np.ascontiguousarray(
        x.T.reshape(D // 128, 128, seq // 512, 512).transpose(1, 2, 0, 3)
    ).astype(bf16)

    pos = np.asarray(position_ids).reshape(seq).astype(np.float32)
    invf = (1.0 / (BASE ** (np.arange(0, HD, 2, dtype=np.float32) / HD))).astype(
        np.float32
    )
    freqs = pos[:, None] * invf[None, :]  # [seq, 64]
    cos64 = np.cos(freqs).T  # [64, seq]
    sin64 = np.sin(freqs).T
    cosT = np.ascontiguousarray(np.concatenate([cos64, cos64], axis=0)).astype(bf16)
    sinTs = np.ascontiguousarray(np.concatenate([sin64, -sin64], axis=0)).astype(bf16)

    Wq = np.asarray(Wq, dtype=np.float32)
    Wk = np.asarray(Wk, dtype=np.float32)
    Wv = np.asarray(Wv, dtype=np.float32)
    Wo = np.asarray(Wo, dtype=np.float32)

    in_maps = []
    for c in range(N_CORES):
        r = slice(c * HPC * HD, (c + 1) * HPC * HD)

        def wshard(W):
            # [D, 256] -> [128, 16, 256] with [p, o, e] = W[r][e, o*128+p]
            wt = W[r, :].T  # [D, 256]
            return np.ascontiguousarray(
                wt.reshape(D // 128, 128, HPC * HD).transpose(1, 0, 2)
            ).astype(bf16)

        woc = Wo[:, r].T  # [256, D]
        woc = np.ascontiguousarray(
            woc.reshape(HPC, HD, D).transpose(1, 0, 2)
        ).astype(np.float32)
        in_maps.append(
            {
                "xT": xT,
                "wq": wshard(Wq),
                "wk": wshard(Wk),
                "wv": wshard(Wv),
                "wo": woc,
                "cosT": cosT,
                "sinTs": sinTs,
            }
        )
    return in_maps


_NC_CACHE = {}


def get_nc(seq=S):
    if seq not in _NC_CACHE:
        _NC_CACHE[seq] = build_nc(seq)
    return _NC_CACHE[seq]


def unstage(arr, seq=S):
    return np.asarray(arr)


def kernel(hidden_states, Wq, Wk, Wv, Wo, position_ids):
    nc = get_nc()
    in_maps = make_in_maps(hidden_states, Wq, Wk, Wv, Wo, position_ids)
    res = run_bass_kernel_spmd(nc, in_maps, core_ids=list(range(N_CORES)))
    out = np.zeros((S, D), dtype=np.float32)
    for r in res.results:
        out += unstage(r["outp"])
    return out.reshape(1, S, D)



# revision 7
# speedup vs baseline: 2.0757x; 2.0757x over previous
"""Trainium2 Bass kernel for a single dense-transformer attention layer.

Problem (hardcoded): B=1, S=4096, D=2048, H=16 heads, head_dim=128, RoPE,
softmax attention, output projection.  torch-Linear convention: y = x @ W.T.

Sharding: tensor-parallel over heads across 8 NeuronCores.  Each core handles
2 heads: it computes q/k/v projections for its head slice, RoPE, attention,
and a partial output projection (contraction over its 256 head-dims of Wo).
The host sums the 8 partial [S, D] outputs.

Device-side layout choices (everything pre-arranged on host):
  - xT      [128, 16, S]  bf16 : xT[p, o, s] = x[s, o*128+p]      (x transposed)
  - wq/wk/wv[128, 16, 256] bf16: w[p, o, e]  = W[c*256+e, o*128+p] (per core c)
  - wo      [128, 2, D]   bf16 : wo[p, h, n] = Wo[n, (2c+h)*128+p]
  - cosT    [128, S] bf16 : cosT[d, s]  = cos(pos_s * invfreq(d % 64))
  - sinTs   [128, S] bf16 : +sin for d<64, -sin for d>=64 (pre-shifted table
                            so RoPE needs only aligned multiplies + a
                            64-partition-shift copy, done by DMA)

The q/k projections are emitted transposed ([head_dim, s]) so the scores
matmul can contract over head_dim directly; scores are computed transposed
([s_k, s_q]) so exp(scores) feeds the P@V matmul with V in natural layout and
no on-chip transposes anywhere.  The softmax denominator comes from a ones
column vector matmul riding the same exp tiles; normalization is folded into
the PSUM->SBUF copy of the attention output via a broadcast reciprocal.
"""

import os
import numpy as np
import ml_dtypes

import concourse.bacc as bacc
import concourse.bass as bass
import concourse.mybir as mybir
import concourse.tile as tile
from concourse.bass import ds, ts
from concourse.bass_utils import run_bass_kernel_spmd

BF16 = mybir.dt.bfloat16
F32 = mybir.dt.float32
F32R = mybir.dt.float32r
AF = mybir.ActivationFunctionType

S, D, H, HD = 4096, 2048, 16, 128
N_CORES = 8
HPC = H // N_CORES  # heads per core = 2
BASE = 10000.0


def build_nc(seq=S, repeat=1):
    """Build the per-core Bass module (identical program on all 8 cores)."""
    n_strips = seq // 512  # 512-wide s strips
    n_ktiles = seq // 128  # 128-wide k chunks
    DK = D // 128  # 16 contraction chunks for projections

    nc = bacc.Bacc("TRN2", target_bir_lowering=False)

    xT = nc.dram_tensor("xT", [128, seq // 512, DK, 512], BF16, kind="ExternalInput")
    wq = nc.dram_tensor("wq", [128, DK, HPC * HD], BF16, kind="ExternalInput")
    wk = nc.dram_tensor("wk", [128, DK, HPC * HD], BF16, kind="ExternalInput")
    wv = nc.dram_tensor("wv", [128, DK, HPC * HD], BF16, kind="ExternalInput")
    wo = nc.dram_tensor("wo", [128, HPC, D], F32, kind="ExternalInput")
    cosT = nc.dram_tensor("cosT", [128, seq], BF16, kind="ExternalInput")
    sinTs = nc.dram_tensor("sinTs", [128, seq], BF16, kind="ExternalInput")
    outp = nc.dram_tensor("outp", [seq, D], F32, kind="ExternalOutput")

    inv_sqrt_hd = 1.0 / float(np.sqrt(HD))

    with tile.TileContext(nc) as tc:
        from contextlib import ExitStack

        with ExitStack() as ctx:
            cpool = ctx.enter_context(tc.tile_pool(name="const", bufs=1))
            qkpool = ctx.enter_context(tc.tile_pool(name="qk", bufs=1))
            vpool = ctx.enter_context(tc.tile_pool(name="v", bufs=1))

            cos_sb = cpool.tile([128, seq], BF16, tag="cos")
            nc.sync.dma_start(cos_sb[:], cosT[:])
            sin_sb = cpool.tile([128, seq], BF16, tag="sin")
            nc.sync.dma_start(sin_sb[:], sinTs[:])
            wq_sb = cpool.tile([128, DK, HPC * HD], BF16, tag="wq")
            nc.sync.dma_start(wq_sb[:], wq[:])
            wk_sb = cpool.tile([128, DK, HPC * HD], BF16, tag="wk")
            nc.sync.dma_start(wk_sb[:], wk[:])
            wv_sb = cpool.tile([128, DK, HPC * HD], BF16, tag="wv")
            nc.sync.dma_start(wv_sb[:], wv[:])
            wo_sb = cpool.tile([128, HPC, D], F32R, tag="wo")
            with ExitStack() as wstage_ctx:
                wpool = wstage_ctx.enter_context(tc.tile_pool(name="wstage", bufs=1))
                wo_raw = wpool.tile([128, HPC, D], F32, tag="wo_raw")
                nc.sync.dma_start(wo_raw[:], wo[:])
                nc.vector.tensor_copy(wo_sb[:], wo_raw[:])
            ones_stage = cpool.tile([128, 1], F32, tag="ones_stage")
            nc.vector.memset(ones_stage[:], 1.0)
            ones_col = cpool.tile([128, 1], F32R, tag="ones_col")
            nc.vector.tensor_copy(ones_col[:], ones_stage[:])
            ones_row_stage = cpool.tile([1, 128], F32, tag="ones_row_stage")
            nc.vector.memset(ones_row_stage[:], 1.0)
            ones_row = cpool.tile([1, 128], F32R, tag="ones_row")
            nc.vector.tensor_copy(ones_row[:], ones_row_stage[:])

            # persistent per-head activations (bf16)
            q_sb = [qkpool.tile([128, seq], F32R, tag=f"q{h}", name=f"q{h}") for h in range(HPC)]
            k_sb = [qkpool.tile([128, seq], F32R, tag=f"k{h}", name=f"k{h}") for h in range(HPC)]
            v_sb = [
                vpool.tile([128, n_ktiles, HD], F32R, tag=f"v{h}", name=f"v{h}") for h in range(HPC)
            ]

            for _rep in range(repeat):
                # ---------------- Phase 1: QKV projections + RoPE ----------------
                with ExitStack() as p1:
                    xpool = p1.enter_context(tc.tile_pool(name="xchunk", bufs=2))
                    rpool = p1.enter_context(tc.tile_pool(name="rope", bufs=4))
                    ps_qk = p1.enter_context(
                        tc.tile_pool(name="ps_qk", bufs=6, space="PSUM")
                    )
                    ps_v = p1.enter_context(tc.tile_pool(name="ps_v", bufs=2, space="PSUM"))

                    for j in range(n_strips):
                        xc = xpool.tile([128, DK, 512], BF16, tag="xc")
                        nc.sync.dma_start(xc[:], xT[:, j])
                        for h in range(HPC):
                            for w_sb, dst in ((wq_sb, q_sb[h]), (wk_sb, k_sb[h])):
                                ps = ps_qk.tile([128, 512], F32, tag="ps_qk")
                                for o in range(DK):
                                    nc.tensor.matmul(
                                        ps[:],
                                        lhsT=w_sb[:, o, ts(h, HD)],
                                        rhs=xc[:, o, :],
                                        start=(o == 0),
                                        stop=(o == DK - 1),
                                    )
                                # RoPE: dst = ps*cos + shift64(ps*sinTs)
                                mp = rpool.tile([128, 512], BF16, tag="mp")
                                nc.vector.tensor_mul(mp[:], ps[:], sin_sb[:, ts(j, 512)])
                                m = rpool.tile([128, 512], BF16, tag="m")
                                nc.sync.dma_start(m[0:64, :], mp[64:128, :])
                                nc.sync.dma_start(m[64:128, :], mp[0:64, :])
                                tt = rpool.tile([128, 512], F32, tag="tt")
                                nc.vector.tensor_mul(tt[:], ps[:], cos_sb[:, ts(j, 512)])
                                nc.vector.tensor_add(dst[:, ts(j, 512)], tt[:], m[:])
                        for b in range(4):  # v in natural layout, both heads at once
                            sblk = j * 4 + b
                            psv = ps_v.tile([128, HPC * HD], F32, tag="psv")
                            for o in range(DK):
                                nc.tensor.matmul(
                                    psv[:],
                                    lhsT=xc[:, o, ts(b, 128)],
                                    rhs=wv_sb[:, o, :],
                                    start=(o == 0),
                                    stop=(o == DK - 1),
                                )
                            for h in range(HPC):
                                nc.scalar.copy(v_sb[h][:, sblk, :], psv[:, ts(h, HD)])

                # ---------------- Phase 2: attention + output projection --------
                with ExitStack() as p2:
                    epool = p2.enter_context(tc.tile_pool(name="et", bufs=4))
                    opool = p2.enter_context(tc.tile_pool(name="outsb", bufs=3))
                    apool = p2.enter_context(tc.tile_pool(name="attnT", bufs=2))
                    rcpool = p2.enter_context(tc.tile_pool(name="rc", bufs=2))
                    ps_s = p2.enter_context(tc.tile_pool(name="ps_s", bufs=2, space="PSUM"))
                    ps_pv = p2.enter_context(
                        tc.tile_pool(name="ps_pv", bufs=2, space="PSUM")
                    )
                    ps_rs = p2.enter_context(
                        tc.tile_pool(name="ps_rs", bufs=2, space="PSUM")
                    )
                    ps_bc = p2.enter_context(
                        tc.tile_pool(name="ps_bc", bufs=1, space="PSUM")
                    )
                    ps_o = p2.enter_context(tc.tile_pool(name="ps_o", bufs=1, space="PSUM"))

                    for j in range(n_strips):
                        aT = apool.tile([128, HPC, 512], F32R, tag="aT")
                        pvs, rss = [], []
                        for h in range(HPC):
                            pv = ps_pv.tile([128, 512], F32, tag="pv", name=f"pv{h}")
                            rs = ps_rs.tile([1, 512], F32, tag="rs", name=f"rs{h}")
                            pvs.append(pv)
                            rss.append(rs)
                            for c in range(n_ktiles):
                                # transposed scores chunk: [s_k 128, s_q 512]
                                ss = ps_s.tile([128, 512], F32, tag="ss")
                                nc.tensor.matmul(
                                    ss[:],
                                    lhsT=k_sb[h][:, ts(c, 128)],
                                    rhs=q_sb[h][:, ts(j, 512)],
                                    start=True,
                                    stop=True,
                                )
                                et = epool.tile([128, 512], F32R, tag="et")
                                nc.scalar.activation(
                                    et[:], ss[:], AF.Exp, bias=0.0, scale=inv_sqrt_hd
                                )
                                nc.tensor.matmul(
                                    pv[:],
                                    lhsT=v_sb[h][:, c, :],
                                    rhs=et[:],
                                    start=(c == 0),
                                    stop=(c == n_ktiles - 1),
                                )
                                nc.tensor.matmul(
                                    rs[:],
                                    lhsT=ones_col[:],
                                    rhs=et[:],
                                    start=(c == 0),
                                    stop=(c == n_ktiles - 1),
                                )
                        # normalization for both heads, after both chunk loops so
                        # the 64 exps above form one uninterrupted ACT run
                        for h in range(HPC):
                            rcp = rcpool.tile([1, 512], F32R, tag="rcp")
                            with nc.allow_low_precision(reason="fp32r rounding only"):
                                nc.vector.reciprocal(rcp[:], rss[h][:])
                            bc = ps_bc.tile([128, 512], F32, tag="bc")
                            nc.tensor.matmul(
                                bc[:],
                                lhsT=ones_row[:],
                                rhs=rcp[:],
                                start=True,
                                stop=True,
                            )
                            bcs = rcpool.tile([128, 512], F32, tag="bcs")
                            nc.vector.tensor_copy(bcs[:], bc[:])
                            nc.vector.tensor_mul(aT[:, h, :], pvs[h][:], bcs[:])
                        # output projection for the strip's 4 q-tiles
                        for b in range(4):
                            ob = opool.tile([128, D], F32, tag="ob")
                            for n in range(D // 512):
                                po = ps_o.tile([128, 512], F32, tag="po")
                                for h in range(HPC):
                                    nc.tensor.matmul(
                                        po[:],
                                        lhsT=aT[:, h, ts(b, 128)],
                                        rhs=wo_sb[:, h, ts(n, 512)],
                                        start=(h == 0),
                                        stop=(h == HPC - 1),
                                    )
                                nc.vector.tensor_copy(ob[:, ts(n, 512)], po[:])
                            nc.sync.dma_start(outp[ds((j * 4 + b) * 128, 128), :], ob[:])

    nc.compile()
    return nc


def make_in_maps(hidden_states, Wq, Wk, Wv, Wo, position_ids, seq=S):
    """Host-side prep: transpose/shard/cast inputs into per-core in_maps."""
    bf16 = ml_dtypes.bfloat16
    x = np.asarray(hidden_states, dtype=np.float32).reshape(seq, D)
    # [p, strip, o, 512] so each strip chunk is contiguous per partition
    xT = np.ascontiguousarray(
        x.T.reshape(D // 128, 128, seq // 512, 512).transpose(1, 2, 0, 3)
    ).astype(bf16)

    pos = np.asarray(position_ids).reshape(seq).astype(np.float32)
    invf = (1.0 / (BASE ** (np.arange(0, HD, 2, dtype=np.float32) / HD))).astype(
        np.float32
    )
    freqs = pos[:, None] * invf[None, :]  # [seq, 64]
    cos64 = np.cos(freqs).T  # [64, seq]
    sin64 = np.sin(freqs).T
    cosT = np.ascontiguousarray(np.concatenate([cos64, cos64], axis=0)).astype(bf16)
    sinTs = np.ascontiguousarray(np.concatenate([sin64, -sin64], axis=0)).astype(bf16)

    Wq = np.asarray(Wq, dtype=np.float32)
    Wk = np.asarray(Wk, dtype=np.float32)
    Wv = np.asarray(Wv, dtype=np.float32)
    Wo = np.asarray(Wo, dtype=np.float32)

    in_maps = []
    for c in range(N_CORES):
        r = slice(c * HPC * HD, (c + 1) * HPC * HD)

        def wshard(W):
            # [D, 256] -> [128, 16, 256] with [p, o, e] = W[r][e, o*128+p]
            wt = W[r, :].T  # [D, 256]
            return np.ascontiguousarray(
                wt.reshape(D // 128, 128, HPC * HD).transpose(1, 0, 2)
            ).astype(bf16)

        woc = Wo[:, r].T  # [256, D]
        woc = np.ascontiguousarray(
            woc.reshape(HPC, HD, D).transpose(1, 0, 2)
        ).astype(np.float32)
        in_maps.append(
            {
                "xT": xT,
                "wq": wshard(Wq),
                "wk": wshard(Wk),
                "wv": wshard(Wv),
                "wo": woc,
                "cosT": cosT,
                "sinTs": sinTs,
            }
        )
    return in_maps


_NC_CACHE = {}


def get_nc(seq=S):
    if seq not in _NC_CACHE:
        _NC_CACHE[seq] = build_nc(seq)
    return _NC_CACHE[seq]


def unstage(arr, seq=S):
    return np.asarray(arr)


def kernel(hidden_states, Wq, Wk, Wv, Wo, position_ids):
    nc = get_nc()
    in_maps = make_in_maps(hidden_states, Wq, Wk, Wv, Wo, position_ids)
    res = run_bass_kernel_spmd(nc, in_maps, core_ids=list(range(N_CORES)))
    out = np.zeros((S, D), dtype=np.float32)
    for r in res.results:
        out += unstage(r["outp"])
    return out.reshape(1, S, D)



# revision 12
# speedup vs baseline: 4.2376x; 2.0415x over previous
"""Trainium2 Bass kernel for a single dense-transformer attention layer.

Problem (hardcoded): B=1, S=4096, D=2048, H=16 heads, head_dim=128, RoPE,
softmax attention, output projection.  torch-Linear convention: y = x @ W.T.

Sharding: tensor-parallel over heads across 8 NeuronCores.  Each core handles
2 heads: it computes q/k/v projections for its head slice, RoPE, attention,
and a partial output projection (contraction over its 256 head-dims of Wo).
The host sums the 8 partial [S, D] outputs.

Device-side layout choices (everything pre-arranged on host):
  - xT      [128, 16, S]  bf16 : xT[p, o, s] = x[s, o*128+p]      (x transposed)
  - wq/wk/wv[128, 16, 256] bf16: w[p, o, e]  = W[c*256+e, o*128+p] (per core c)
  - wo      [128, 2, D]   bf16 : wo[p, h, n] = Wo[n, (2c+h)*128+p]
  - cosT    [128, S] bf16 : cosT[d, s]  = cos(pos_s * invfreq(d % 64))
  - sinTs   [128, S] bf16 : +sin for d<64, -sin for d>=64 (pre-shifted table
                            so RoPE needs only aligned multiplies + a
                            64-partition-shift copy, done by DMA)

The q/k projections are emitted transposed ([head_dim, s]) so the scores
matmul can contract over head_dim directly; scores are computed transposed
([s_k, s_q]) so exp(scores) feeds the P@V matmul with V in natural layout and
no on-chip transposes anywhere.  The softmax denominator comes from a ones
column vector matmul riding the same exp tiles; normalization is folded into
the PSUM->SBUF copy of the attention output via a broadcast reciprocal.
"""

import os
import numpy as np
import ml_dtypes

import concourse.bacc as bacc
import concourse.bass as bass
import concourse.mybir as mybir
import concourse.tile as tile
from concourse.bass import ds, ts
from concourse.bass_utils import run_bass_kernel_spmd

BF16 = mybir.dt.bfloat16
F32 = mybir.dt.float32
F32R = mybir.dt.float32r
AF = mybir.ActivationFunctionType

S, D, H, HD = 4096, 2048, 16, 128
N_CORES = 8
HPC = H // N_CORES  # heads per core = 2
BASE = 10000.0


def build_nc(seq=S, repeat=1):
    """Build the per-core Bass module (identical program on all 8 cores)."""
    n_strips = seq // 512  # 512-wide s strips
    n_ktiles = seq // 128  # 128-wide k chunks
    DK = D // 128  # 16 contraction chunks for projections

    nc = bacc.Bacc("TRN2", target_bir_lowering=False)

    xT = nc.dram_tensor("xT", [128, seq // 512, DK, 512], BF16, kind="ExternalInput")
    wq = nc.dram_tensor("wq", [128, DK, HPC * HD], BF16, kind="ExternalInput")
    wk = nc.dram_tensor("wk", [128, DK, HPC * HD], BF16, kind="ExternalInput")
    wv = nc.dram_tensor("wv", [128, DK, HPC * HD], BF16, kind="ExternalInput")
    wo = nc.dram_tensor("wo", [128, HPC, D], F32R, kind="ExternalInput")
    cosT = nc.dram_tensor("cosT", [128, seq], BF16, kind="ExternalInput")
    sinTs = nc.dram_tensor("sinTs", [128, seq], BF16, kind="ExternalInput")
    outp = nc.dram_tensor("outp", [seq, D], F32, kind="ExternalOutput")

    inv_sqrt_hd = 1.0 / float(np.sqrt(HD))

    with tile.TileContext(nc) as tc:
        from contextlib import ExitStack

        with ExitStack() as ctx:
            cpool = ctx.enter_context(tc.tile_pool(name="const", bufs=1))
            qkpool = ctx.enter_context(tc.tile_pool(name="qk", bufs=1))
            vpool = ctx.enter_context(tc.tile_pool(name="v", bufs=1))

            # weights for the first matmuls come first in the DMA queue
            wq_sb = cpool.tile([128, DK, HPC * HD], BF16, tag="wq")
            nc.sync.dma_start(wq_sb[:], wq[:])
            wk_sb = cpool.tile([128, DK, HPC * HD], BF16, tag="wk")
            nc.sync.dma_start(wk_sb[:], wk[:])
            # cos/sin/wv/wo DMAs are deferred into the phase-1 loop so the
            # first x strip wins the DMA queue; wo (phase 2 only) goes last
            cos_sb = cpool.tile([128, seq], BF16, tag="cos")
            sin_sb = cpool.tile([128, seq], BF16, tag="sin")
            wv_sb = cpool.tile([128, DK, HPC * HD], BF16, tag="wv")
            wo_sb = cpool.tile([128, HPC, D], F32R, tag="wo")
            ones_stage = cpool.tile([128, 1], F32, tag="ones_stage")
            nc.vector.memset(ones_stage[:], 1.0)
            ones_col = cpool.tile([128, 1], F32R, tag="ones_col")
            nc.vector.tensor_copy(ones_col[:], ones_stage[:])
            ones_row_stage = cpool.tile([1, 128], F32, tag="ones_row_stage")
            nc.vector.memset(ones_row_stage[:], 1.0)
            ones_row = cpool.tile([1, 128], F32R, tag="ones_row")
            nc.vector.tensor_copy(ones_row[:], ones_row_stage[:])

            # persistent per-head activations (bf16)
            q_sb = [qkpool.tile([128, seq], F32R, tag=f"q{h}", name=f"q{h}") for h in range(HPC)]
            k_sb = [qkpool.tile([128, seq], F32R, tag=f"k{h}", name=f"k{h}") for h in range(HPC)]
            v_sb = [
                vpool.tile([128, n_ktiles, HD], F32R, tag=f"v{h}", name=f"v{h}") for h in range(HPC)
            ]

            for _rep in range(repeat):
                # ---------------- Phase 1: QKV projections + RoPE ----------------
                with ExitStack() as p1:
                    xpool = p1.enter_context(tc.tile_pool(name="xchunk", bufs=2))
                    rpool = p1.enter_context(tc.tile_pool(name="rope", bufs=4))
                    ps_qk = p1.enter_context(
                        tc.tile_pool(name="ps_qk", bufs=6, space="PSUM")
                    )
                    ps_v = p1.enter_context(tc.tile_pool(name="ps_v", bufs=2, space="PSUM"))

                    for j in range(n_strips):
                        xc = xpool.tile([128, DK, 512], BF16, tag="xc")
                        nc.sync.dma_start(xc[:], xT[:, j])
                        if j == 0 and _rep == 0:
                            nc.sync.dma_start(cos_sb[:], cosT[:])
                            nc.sync.dma_start(sin_sb[:], sinTs[:])
                            nc.sync.dma_start(wv_sb[:], wv[:])
                        if j == min(1, n_strips - 1) and _rep == 0:
                            nc.sync.dma_start(wo_sb[:], wo[:])
                        for h in range(HPC):
                            for w_sb, dst in ((wq_sb, q_sb[h]), (wk_sb, k_sb[h])):
                                ps = ps_qk.tile([128, 512], F32, tag="ps_qk")
                                for o in range(DK):
                                    nc.tensor.matmul(
                                        ps[:],
                                        lhsT=w_sb[:, o, ts(h, HD)],
                                        rhs=xc[:, o, :],
                                        start=(o == 0),
                                        stop=(o == DK - 1),
                                    )
                                # RoPE: dst = ps*cos + shift64(ps*sinTs)
                                mp = rpool.tile([128, 512], BF16, tag="mp")
                                nc.vector.tensor_mul(mp[:], ps[:], sin_sb[:, ts(j, 512)])
                                m = rpool.tile([128, 512], BF16, tag="m")
                                nc.sync.dma_start(m[0:64, :], mp[64:128, :])
                                nc.sync.dma_start(m[64:128, :], mp[0:64, :])
                                tt = rpool.tile([128, 512], F32, tag="tt")
                                nc.vector.tensor_mul(tt[:], ps[:], cos_sb[:, ts(j, 512)])
                                nc.vector.tensor_add(dst[:, ts(j, 512)], tt[:], m[:])
                        for b in range(4):  # v in natural layout, both heads at once
                            sblk = j * 4 + b
                            psv = ps_v.tile([128, HPC * HD], F32, tag="psv")
                            for o in range(DK):
                                nc.tensor.matmul(
                                    psv[:],
                                    lhsT=xc[:, o, ts(b, 128)],
                                    rhs=wv_sb[:, o, :],
                                    start=(o == 0),
                                    stop=(o == DK - 1),
                                )
                            for h in range(HPC):
                                nc.scalar.copy(v_sb[h][:, sblk, :], psv[:, ts(h, HD)])

                # ---------------- Phase 2: attention + output projection --------
                with ExitStack() as p2:
                    epool = p2.enter_context(tc.tile_pool(name="et", bufs=6))
                    opool = p2.enter_context(tc.tile_pool(name="outsb", bufs=3))
                    apool = p2.enter_context(tc.tile_pool(name="attnT", bufs=2))
                    rcpool = p2.enter_context(tc.tile_pool(name="rc", bufs=2))
                    ps_s = p2.enter_context(tc.tile_pool(name="ps_s", bufs=3, space="PSUM"))
                    ps_pv = p2.enter_context(
                        tc.tile_pool(name="ps_pv", bufs=2, space="PSUM")
                    )
                    ps_rs = p2.enter_context(
                        tc.tile_pool(name="ps_rs", bufs=1, space="PSUM")
                    )
                    ps_o = p2.enter_context(tc.tile_pool(name="ps_o", bufs=2, space="PSUM"))

                    for j in range(n_strips):
                        aT = apool.tile([128, HPC, 512], F32R, tag="aT")
                        for h in range(HPC):
                            pv = ps_pv.tile([128, 512], F32, tag="pv", name=f"pv{h}")
                            rs = ps_rs.tile([1, 512], F32, tag="rs", name=f"rs{h}")
                            # scores+exp run LOOKAHEAD chunks ahead of pv/rs so
                            # the PE never waits on the exp semaphore round-trip
                            et_t = {}

                            def emit_ss(c, h=h, j=j, et_t=et_t):
                                ss = ps_s.tile([128, 512], F32, tag="ss")
                                nc.tensor.matmul(
                                    ss[:],
                                    lhsT=k_sb[h][:, ts(c, 128)],
                                    rhs=q_sb[h][:, ts(j, 512)],
                                    start=True,
                                    stop=True,
                                )
                                et = epool.tile([128, 512], F32R, tag="et")
                                nc.scalar.activation(
                                    et[:], ss[:], AF.Exp, bias=0.0, scale=inv_sqrt_hd
                                )
                                et_t[c] = et

                            LOOKAHEAD = 2
                            for c in range(LOOKAHEAD):
                                emit_ss(c)
                            for c in range(n_ktiles):
                                if c + LOOKAHEAD < n_ktiles:
                                    emit_ss(c + LOOKAHEAD)
                                et = et_t.pop(c)
                                nc.tensor.matmul(
                                    pv[:],
                                    lhsT=v_sb[h][:, c, :],
                                    rhs=et[:],
                                    start=(c == 0),
                                    stop=(c == n_ktiles - 1),
                                )
                                nc.tensor.matmul(
                                    rs[:],
                                    lhsT=ones_col[:],
                                    rhs=et[:],
                                    start=(c == 0),
                                    stop=(c == n_ktiles - 1),
                                )
                            # normalize this head right away; overlaps the other
                            # head's chunk loop and frees rs/pv banks early
                            rcp = rcpool.tile([1, 512], F32, tag="rcp")
                            nc.vector.reciprocal(rcp[:], rs[:])
                            bcs = rcpool.tile([128, 512], F32, tag="bcs")
                            nc.gpsimd.partition_broadcast(bcs[:], rcp[:])
                            nc.vector.tensor_mul(aT[:, h, :], pv[:], bcs[:])
                        # output projection for the strip's 4 q-tiles
                        for b in range(4):
                            ob = opool.tile([128, D], F32, tag="ob")
                            for n in range(D // 512):
                                po = ps_o.tile([128, 512], F32, tag="po")
                                for h in range(HPC):
                                    nc.tensor.matmul(
                                        po[:],
                                        lhsT=aT[:, h, ts(b, 128)],
                                        rhs=wo_sb[:, h, ts(n, 512)],
                                        start=(h == 0),
                                        stop=(h == HPC - 1),
                                    )
                                if n % 2 == 0:
                                    nc.scalar.copy(ob[:, ts(n, 512)], po[:])
                                else:
                                    nc.vector.tensor_copy(ob[:, ts(n, 512)], po[:])
                            nc.sync.dma_start(outp[ds((j * 4 + b) * 128, 128), :], ob[:])

    nc.compile()
    return nc


def make_in_maps(hidden_states, Wq, Wk, Wv, Wo, position_ids, seq=S):
    """Host-side prep: transpose/shard/cast inputs into per-core in_maps."""
    bf16 = ml_dtypes.bfloat16
    x = np.asarray(hidden_states, dtype=np.float32).reshape(seq, D)
    # [p, strip, o, 512] so each strip chunk is contiguous per partition
    xT = np.ascontiguousarray(
        x.T.reshape(D // 128, 128, seq // 512, 512).transpose(1, 2, 0, 3)
    ).astype(bf16)

    pos = np.asarray(position_ids).reshape(seq).astype(np.float32)
    invf = (1.0 / (BASE ** (np.arange(0, HD, 2, dtype=np.float32) / HD))).astype(
        np.float32
    )
    freqs = pos[:, None] * invf[None, :]  # [seq, 64]
    cos64 = np.cos(freqs).T  # [64, seq]
    sin64 = np.sin(freqs).T
    cosT = np.ascontiguousarray(np.concatenate([cos64, cos64], axis=0)).astype(bf16)
    sinTs = np.ascontiguousarray(np.concatenate([sin64, -sin64], axis=0)).astype(bf16)

    Wq = np.asarray(Wq, dtype=np.float32)
    Wk = np.asarray(Wk, dtype=np.float32)
    Wv = np.asarray(Wv, dtype=np.float32)
    Wo = np.asarray(Wo, dtype=np.float32)

    in_maps = []
    for c in range(N_CORES):
        r = slice(c * HPC * HD, (c + 1) * HPC * HD)

        def wshard(W):
            # [D, 256] -> [128, 16, 256] with [p, o, e] = W[r][e, o*128+p]
            wt = W[r, :].T  # [D, 256]
            return np.ascontiguousarray(
                wt.reshape(D // 128, 128, HPC * HD).transpose(1, 0, 2)
            ).astype(bf16)

        woc = Wo[:, r].T  # [256, D]
        woc = np.ascontiguousarray(
            woc.reshape(HPC, HD, D).transpose(1, 0, 2)
        ).astype(np.float32)
        in_maps.append(
            {
                "xT": xT,
                "wq": wshard(Wq),
                "wk": wshard(Wk),
                "wv": wshard(Wv),
                "wo": woc,
                "cosT": cosT,
                "sinTs": sinTs,
            }
        )
    return in_maps


_NC_CACHE = {}


def get_nc(seq=S):
    if seq not in _NC_CACHE:
        _NC_CACHE[seq] = build_nc(seq)
    return _NC_CACHE[seq]


def unstage(arr, seq=S):
    return np.asarray(arr)


def kernel(hidden_states, Wq, Wk, Wv, Wo, position_ids):
    nc = get_nc()
    in_maps = make_in_maps(hidden_states, Wq, Wk, Wv, Wo, position_ids)
    res = run_bass_kernel_spmd(nc, in_maps, core_ids=list(range(N_CORES)))
    out = np.zeros((S, D), dtype=np.float32)
    for r in res.results:
        out += unstage(r["outp"])
    return out.reshape(1, S, D)

